# revision 6
# baseline (speedup 1.0000x reference)
"""HGRN forward on 8 trn2 NeuronCores.

Sharding: core c = (b, q) with b = c//4 (batch), q = c%4 (T-quarter, 512 tokens).
Layers run T-local in [D-partition, T-free] layout; the linear recurrence
h_t = f_t*h_{t-1} + i_t*c_t is one tensor_tensor_scan per 128-channel tile.
Cross-core scan carries via a tiny AllGather ([[0-3],[4-7]]) + masked combine
+ fixup (h += cumprod_f * h_in).  After LayerNorm, x is AllGathered over all
8 cores and the head matmul is vocab-sharded (4000 cols/core).
Host does layout prep only: embedding gather, weight retiling, Whead transpose.
"""
import sys
sys.path.insert(0, "/opt/trn_rl_repo")
import numpy as np

L, D, B, T, V = 4, 1024, 2, 2048, 32000
NCORES = 8
TLOC = 512            # tokens per core
KT = D // 128         # 8 partition tiles over D
VSH = V // NCORES     # 4000 vocab cols per core
VC = 500              # vocab chunk (<=512 psum free dim)
NVC = VSH // VC       # 8 chunks
NTT = (B * T) // 128  # 32 t-tiles of 128 over all 4096 tokens

_CACHED = {}


def build_nc(loop_reps=False):
    from concourse import bacc, tile, mybir

    f32 = mybir.dt.float32
    Alu = mybir.AluOpType
    Act = mybir.ActivationFunctionType

    nc = bacc.Bacc("TRN2", target_bir_lowering=False, debug=False,
                   num_devices=NCORES)

    x0_in = nc.dram_tensor("x0", [D, TLOC], f32, kind="ExternalInput").ap()
    wi_in = nc.dram_tensor("wi", [L, KT, 128, KT, 128], f32, kind="ExternalInput").ap()
    wf_in = nc.dram_tensor("wf", [L, KT, 128, KT, 128], f32, kind="ExternalInput").ap()
    wg_in = nc.dram_tensor("wg", [L, KT, 128, KT, 128], f32, kind="ExternalInput").ap()
    wo_in = nc.dram_tensor("wo", [L, KT, 128, KT, 128], f32, kind="ExternalInput").ap()
    bf_in = nc.dram_tensor("bf", [L, 128, KT], f32, kind="ExternalInput").ap()
    bg_in = nc.dram_tensor("bg", [L, 128, KT], f32, kind="ExternalInput").ap()
    gb_in = nc.dram_tensor("gb", [2, 128, KT], f32, kind="ExternalInput").ap()
    sel_in = nc.dram_tensor("sel", [128, 4], f32, kind="ExternalInput").ap()
    bf16 = mybir.dt.bfloat16
    wh_in = nc.dram_tensor("wh", [NVC, 128, KT, VC], bf16, kind="ExternalInput").ap()
    out_d = nc.dram_tensor("out", [B * T, VSH], f32, kind="ExternalOutput").ap()
    if loop_reps:
        reps_in = nc.dram_tensor("reps", [1, 1], mybir.dt.int32,
                                 kind="ExternalInput").ap()

    with tile.TileContext(nc) as tc:
        from contextlib import ExitStack
        es = ExitStack()
        sb = es.enter_context(tc.tile_pool(name="sb", bufs=1))
        xp = es.enter_context(tc.tile_pool(name="xp", bufs=12))
        wp = es.enter_context(tc.tile_pool(name="wp", bufs=4))
        ap_ = es.enter_context(tc.tile_pool(name="ap", bufs=2))
        Fp = es.enter_context(tc.tile_pool(name="Fp", bufs=8))
        hp = es.enter_context(tc.tile_pool(name="hp", bufs=8))
        xallp = es.enter_context(tc.tile_pool(name="xallp", bufs=8))
        whp = es.enter_context(tc.tile_pool(name="whp", bufs=2))
        obp = es.enter_context(tc.tile_pool(name="obp", bufs=3))
        psp = es.enter_context(tc.tile_pool(name="psp", bufs=2, space="PSUM"))
        pshead = es.enter_context(tc.tile_pool(name="pshead", bufs=2, space="PSUM"))
        psln = es.enter_context(tc.tile_pool(name="psln", bufs=2, space="PSUM"))
        dramp = es.enter_context(tc.tile_pool(name="dramp", bufs=1, space="DRAM"))

        # DRAM bounce buffers for collectives
        carry_loc = dramp.tile([2, D], f32)
        carry_ag = dramp.tile([4, 2, D], f32)
        x_loc = dramp.tile([D, TLOC], bf16)
        x_ag = dramp.tile([NCORES, D, TLOC], bf16, addr_space="Shared")

        def body():
            # ---- constants / small tiles ----
            zeros = sb.tile([128, TLOC], f32, name="zeros")
            nc.vector.memset(zeros[:], 0.0)
            ones_col = sb.tile([128, 1], f32, name="ones_col")
            nc.vector.memset(ones_col[:], 1.0)
            ones_row = sb.tile([1, 128], f32, name="ones_row")
            nc.vector.memset(ones_row[:], 1.0)
            sel_t = sb.tile([128, 4], f32, name="sel_t")
            nc.sync.dma_start(sel_t[:], sel_in[:])
            gb_t = sb.tile([128, 2 * KT], f32, name="gb_t")
            nc.sync.dma_start(gb_t[:, 0:KT], gb_in[0])
            nc.sync.dma_start(gb_t[:, KT:2 * KT], gb_in[1])

            # ---- x0 ----
            x = []
            for k in range(KT):
                xt = xp.tile([128, TLOC], f32, name=f"x_{k}", tag="x")
                nc.sync.dma_start(xt[:], x0_in[k * 128:(k + 1) * 128, :])
                x.append(xt)

            # ---- layers ----
            for l in range(L):
                bias_t = sb.tile([128, 2 * KT], f32, name=f"bias_{l}", tag="bias",
                                 bufs=2)
                nc.sync.dma_start(bias_t[:, 0:KT], bf_in[l])
                nc.sync.dma_start(bias_t[:, KT:2 * KT], bg_in[l])

                Fs, hs = [], []
                for e in range(KT):
                    # candidate: silu(x @ Wi)
                    wt = wp.tile([128, D], f32, name=f"wi_{l}_{e}", tag="w")
                    nc.sync.dma_start(wt[:], wi_in[l, e])
                    ps = psp.tile([128, TLOC], f32, name=f"psc_{l}_{e}", tag="proj")
                    for k in range(KT):
                        nc.tensor.matmul(ps[:], wt[:, k * 128:(k + 1) * 128], x[k][:],
                                         start=(k == 0), stop=(k == KT - 1))
                    c_t = ap_.tile([128, TLOC], f32, name=f"c_{l}_{e}", tag="c")
                    nc.scalar.activation(c_t[:], ps[:], Act.Silu)

                    # forget gate: sigmoid(x @ Wf + bf)
                    wt = wp.tile([128, D], f32, name=f"wf_{l}_{e}", tag="w")
                    nc.sync.dma_start(wt[:], wf_in[l, e])
                    ps = psp.tile([128, TLOC], f32, name=f"psf_{l}_{e}", tag="proj")
                    for k in range(KT):
                        nc.tensor.matmul(ps[:], wt[:, k * 128:(k + 1) * 128], x[k][:],
                                         start=(k == 0), stop=(k == KT - 1))
                    f_t = ap_.tile([128, TLOC], f32, name=f"f_{l}_{e}", tag="f")
                    nc.scalar.activation(f_t[:], ps[:], Act.Sigmoid,
                                         bias=bias_t[:, e:e + 1])

                    # input gate: sigmoid(x @ Wg + bg)
                    wt = wp.tile([128, D], f32, name=f"wg_{l}_{e}", tag="w")
                    nc.sync.dma_start(wt[:], wg_in[l, e])
                    ps = psp.tile([128, TLOC], f32, name=f"psi_{l}_{e}", tag="proj")
                    for k in range(KT):
                        nc.tensor.matmul(ps[:], wt[:, k * 128:(k + 1) * 128], x[k][:],
                                         start=(k == 0), stop=(k == KT - 1))
                    i_t = ap_.tile([128, TLOC], f32, name=f"i_{l}_{e}", tag="i")
                    nc.scalar.activation(i_t[:], ps[:], Act.Sigmoid,
                                         bias=bias_t[:, KT + e:KT + e + 1])

                    # u = i * c
                    u_t = ap_.tile([128, TLOC], f32, name=f"u_{l}_{e}", tag="u")
                    nc.vector.tensor_mul(u_t[:], i_t[:], c_t[:])

                    # scans: h_loc (vector), F = cumprod f (gpsimd)
                    F_t = Fp.tile([128, TLOC], f32, name=f"F_{l}_{e}", tag="F")
                    nc.vector.tensor_tensor_scan(F_t[:], f_t[:], zeros[:],
                                                 initial=1.0,
                                                 op0=Alu.mult, op1=Alu.add)
                    h_t = hp.tile([128, TLOC], f32, name=f"h_{l}_{e}", tag="h")
                    nc.vector.tensor_tensor_scan(h_t[:], f_t[:], u_t[:],
                                                 initial=0.0,
                                                 op0=Alu.mult, op1=Alu.add)
                    # carry columns to DRAM
                    nc.sync.dma_start(carry_loc[0:1, e * 128:(e + 1) * 128],
                                      F_t[:, TLOC - 1:TLOC])
                    nc.sync.dma_start(carry_loc[1:2, e * 128:(e + 1) * 128],
                                      h_t[:, TLOC - 1:TLOC])
                    Fs.append(F_t)
                    hs.append(h_t)

                # carry exchange within the batch row
                nc.gpsimd.collective_compute(
                    "AllGather", Alu.bypass,
                    replica_groups=[[0, 1, 2, 3], [4, 5, 6, 7]],
                    ins=[carry_loc.opt()], outs=[carry_ag.opt()])

                cw = sb.tile([128, 64], f32, name=f"cw_{l}", tag="cw", bufs=2)
                nc.sync.dma_start(
                    cw[:], carry_ag[:].rearrange("q c (e p) -> p (q c e)", p=128))
                cw4 = cw[:].rearrange("p (q c e) -> p q c e", q=4, c=2, e=KT)
                hin = sb.tile([128, KT], f32, name=f"hin_{l}", tag="hin", bufs=2)
                for e in range(KT):
                    Hpre = sb.tile([128, 4], f32, name=f"Hpre_{l}_{e}", tag="Hpre",
                                   bufs=2)
                    Pv = cw4[:, :, 0, e].squeeze()
                    hv = cw4[:, :, 1, e].squeeze()
                    nc.vector.tensor_tensor_scan(Hpre[:], Pv, hv, initial=0.0,
                                                 op0=Alu.mult, op1=Alu.add)
                    tmp = sb.tile([128, 4], f32, name=f"tmp_{l}_{e}", tag="tmpm",
                                  bufs=2)
                    nc.vector.tensor_mul(tmp[:], Hpre[:], sel_t[:])
                    nc.vector.tensor_reduce(hin[:, e:e + 1], tmp[:],
                                            axis=mybir.AxisListType.X, op=Alu.add)
                # fixup: h = F * h_in + h
                for e in range(KT):
                    nc.vector.scalar_tensor_tensor(hs[e][:], Fs[e][:],
                                                   hin[:, e:e + 1], hs[e][:],
                                                   op0=Alu.mult, op1=Alu.add)

                # output projection
                xn = []
                for e2 in range(KT):
                    wt = wp.tile([128, D], f32, name=f"wo_{l}_{e2}", tag="w")
                    nc.sync.dma_start(wt[:], wo_in[l, e2])
                    ps = psp.tile([128, TLOC], f32, name=f"pso_{l}_{e2}", tag="proj")
                    for k in range(KT):
                        nc.tensor.matmul(ps[:], wt[:, k * 128:(k + 1) * 128],
                                         hs[k][:],
                                         start=(k == 0), stop=(k == KT - 1))
                    xt = xp.tile([128, TLOC], f32, name=f"xn_{l}_{e2}", tag="x")
                    nc.vector.tensor_copy(xt[:], ps[:])
                    xn.append(xt)
                x = xn

            # ---- LayerNorm ----
            ps_mu = psln.tile([1, TLOC], f32, name="ps_mu", tag="lnr")
            for k in range(KT):
                nc.tensor.matmul(ps_mu[:], ones_col[:], x[k][:],
                                 start=(k == 0), stop=(k == KT - 1))
            mu = sb.tile([1, TLOC], f32, name="mu")
            nc.vector.tensor_scalar_mul(mu[:], ps_mu[:], 1.0 / D)

            ps_s2 = psln.tile([1, TLOC], f32, name="ps_s2", tag="lnr")
            for k in range(KT):
                sq = ap_.tile([128, TLOC], f32, name=f"sq_{k}", tag="c")
                nc.scalar.square(sq[:], x[k][:])
                nc.tensor.matmul(ps_s2[:], ones_col[:], sq[:],
                                 start=(k == 0), stop=(k == KT - 1))
            s2 = sb.tile([1, TLOC], f32, name="s2")
            nc.vector.tensor_scalar_mul(s2[:], ps_s2[:], 1.0 / D)
            var = sb.tile([1, TLOC], f32, name="var")
            nc.vector.tensor_mul(var[:], mu[:], mu[:])
            nc.vector.tensor_sub(var[:], s2[:], var[:])
            eps_t = sb.tile([1, 1], f32, name="eps_t")
            nc.vector.memset(eps_t[:], 1e-5)
            std = sb.tile([1, TLOC], f32, name="std")
            nc.scalar.activation(std[:], var[:], Act.Sqrt, bias=eps_t[:])
            rinv = sb.tile([1, TLOC], f32, name="rinv")
            nc.vector.reciprocal(rinv[:], std[:])

            ps_bmu = psln.tile([128, TLOC], f32, name="ps_bmu", tag="lnb")
            nc.tensor.matmul(ps_bmu[:], ones_row[:], mu[:], start=True, stop=True)
            ps_brv = psln.tile([128, TLOC], f32, name="ps_brv", tag="lnb")
            nc.tensor.matmul(ps_brv[:], ones_row[:], rinv[:], start=True, stop=True)
            rb = sb.tile([128, TLOC], f32, name="rb")
            nc.vector.tensor_copy(rb[:], ps_brv[:])

            for k in range(KT):
                t1 = ap_.tile([128, TLOC], f32, name=f"t1_{k}", tag="f")
                nc.vector.tensor_sub(t1[:], x[k][:], ps_bmu[:])
                nc.vector.tensor_mul(t1[:], t1[:], rb[:])
                xh = ap_.tile([128, TLOC], bf16, name=f"xh_{k}", tag="xh", bufs=2)
                nc.vector.tensor_scalar(xh[:], t1[:], gb_t[:, k:k + 1],
                                        gb_t[:, KT + k:KT + k + 1],
                                        op0=Alu.mult, op1=Alu.add)
                nc.sync.dma_start(x_loc[k * 128:(k + 1) * 128, :], xh[:])

            # ---- AllGather x over all 8 cores ----
            nc.gpsimd.collective_compute(
                "AllGather", Alu.bypass,
                replica_groups=[list(range(NCORES))],
                ins=[x_loc.opt()], outs=[x_ag.opt()])

            xall = []
            for k in range(KT):
                xt = xallp.tile([128, B * T], bf16, name=f"xall_{k}", tag="xall")
                for r in range(NCORES):
                    nc.sync.dma_start(xt[:, r * TLOC:(r + 1) * TLOC],
                                      x_ag[r, k * 128:(k + 1) * 128, :])
                xall.append(xt)

            # ---- head ----
            for vc in range(NVC):
                wht = whp.tile([128, KT * VC], bf16, name=f"wh_{vc}", tag="wh")
                nc.sync.dma_start(wht[:], wh_in[vc])
                for tt in range(NTT):
                    ps = pshead.tile([128, VC], f32, name=f"ph_{vc}_{tt}",
                                     tag="head")
                    for k in range(KT):
                        nc.tensor.matmul(ps[:],
                                         xall[k][:, tt * 128:(tt + 1) * 128],
                                         wht[:, k * VC:(k + 1) * VC],
                                         start=(k == 0), stop=(k == KT - 1))
                    ob = obp.tile([128, VC], f32, name=f"ob_{vc}_{tt}", tag="ob")
                    nc.vector.tensor_copy(ob[:], ps[:])
                    nc.sync.dma_start(
                        out_d[tt * 128:(tt + 1) * 128, vc * VC:(vc + 1) * VC],
                        ob[:])

        if loop_reps:
            rt = sb.tile([1, 1], mybir.dt.int32, name="rt")
            nc.sync.dma_start(rt[:], reps_in[:])
            reg = nc.values_load(rt[0:1, 0:1].to_broadcast((1, 1)))
            with tc.For_i(0, reg, 1):
                body()
        else:
            body()
        es.close()

    nc.compile()
    return nc


def prep_inputs(input_ids, emb, Wi, Wf, bf, Wg, bg, Wo, gamma, beta, Whead):
    input_ids = np.asarray(input_ids)
    emb = np.asarray(emb, dtype=np.float32)
    Wi = np.asarray(Wi, dtype=np.float32)
    Wf = np.asarray(Wf, dtype=np.float32)
    bf = np.asarray(bf, dtype=np.float32)
    Wg = np.asarray(Wg, dtype=np.float32)
    bg = np.asarray(bg, dtype=np.float32)
    Wo = np.asarray(Wo, dtype=np.float32)
    gamma = np.asarray(gamma, dtype=np.float32)
    beta = np.asarray(beta, dtype=np.float32)
    Whead = np.asarray(Whead, dtype=np.float32)

    def retile_w(W):
        # [L, D, D] -> [L, e, p, k, m] with W[l, k*128+p, e*128+m]
        return np.ascontiguousarray(
            W.reshape(L, KT, 128, KT, 128).transpose(0, 3, 2, 1, 4))

    wi_p = retile_w(Wi)
    wf_p = retile_w(Wf)
    wg_p = retile_w(Wg)
    wo_p = retile_w(Wo)
    bf_p = np.ascontiguousarray(bf.reshape(L, KT, 128).transpose(0, 2, 1))
    bg_p = np.ascontiguousarray(bg.reshape(L, KT, 128).transpose(0, 2, 1))
    gb_p = np.ascontiguousarray(
        np.stack([gamma, beta]).reshape(2, KT, 128).transpose(0, 2, 1))

    in_maps = []
    for c in range(NCORES):
        b, q = c // 4, c % 4
        tok = input_ids[b, q * TLOC:(q + 1) * TLOC]
        x0 = np.ascontiguousarray(emb[tok].T)  # [D, TLOC]
        sel = np.zeros((128, 4), dtype=np.float32)
        if q > 0:
            sel[:, q - 1] = 1.0
        sh = Whead[c * VSH:(c + 1) * VSH]  # [4000, 1024]
        import ml_dtypes
        wh_p = np.ascontiguousarray(
            sh.reshape(NVC, VC, KT, 128).transpose(0, 3, 2, 1)).astype(
                ml_dtypes.bfloat16)
        in_maps.append({
            "x0": x0, "wi": wi_p, "wf": wf_p, "wg": wg_p, "wo": wo_p,
            "bf": bf_p, "bg": bg_p, "gb": gb_p, "sel": sel, "wh": wh_p,
        })
    return in_maps


def kernel(input_ids, emb, Wi, Wf, bf, Wg, bg, Wo, gamma, beta, Whead):
    from concourse.bass_utils import run_bass_kernel_spmd
    if "nc" not in _CACHED:
        _CACHED["nc"] = build_nc(loop_reps=False)
    nc = _CACHED["nc"]
    in_maps = prep_inputs(input_ids, emb, Wi, Wf, bf, Wg, bg, Wo,
                          gamma, beta, Whead)
    res = run_bass_kernel_spmd(nc, in_maps, list(range(NCORES)))
    outs = [res.results[c]["out"] for c in range(NCORES)]
    logits = np.concatenate(outs, axis=1).reshape(B, T, V)
    return (logits, np.float32(0.0))
